# revision 1
# baseline (speedup 1.0000x reference)
"""GAT message-passing kernel for Trainium2 (8 NeuronCores, batch data-parallel).

out[b,i,:] = sum_j softmax_j(mask(leaky_relu(el_i + er_j))) * h[b,j,:] + x[b,i,:]
  h = x @ W, el = x @ (W a_l), er = x @ (W a_r)
  mask: ADJ_BASE*adj_mask + I > 0.1

Layout: rows (b,n) flattened; tiles of 120 rows = 10 graphs; 8 tiles form one
"super-tile" for the attention elementwise chain ([120, 96] ops).
"""

import numpy as np
import ml_dtypes
from contextlib import ExitStack

import concourse.bass as bass
import concourse.bacc as bacc
import concourse.tile as tile
from concourse import mybir
from concourse.ap import AP
from concourse.bass_utils import run_bass_kernel_spmd
from concourse.bass_test_utils import get_trn_type

N = 12
C = 512
KC = C // 128            # 4 contraction chunks
NEG_SLOPE = 0.2
THRED = 0.1
N_CORES = 8
TILE_R = 120             # rows per matmul tile (10 graphs)
G_PER_TILE = TILE_R // N
ST_TILES = 8             # tiles per super-tile
BF16 = mybir.dt.bfloat16
F32 = mybir.dt.float32
NPBF16 = ml_dtypes.bfloat16

ADJ_BASE = np.array([
    [0,0,0,1,0,1,1,1,1,1,1,1],
    [0,0,0,1,0,1,1,1,1,1,1,1],
    [0,0,0,1,0,1,1,1,1,1,1,1],
    [1,1,1,0,1,1,1,1,1,1,1,1],
    [0,0,0,1,0,1,1,1,1,1,1,1],
    [1,1,1,1,1,0,1,1,1,0,0,0],
    [1,1,1,1,1,1,0,0,0,1,1,1],
    [1,1,1,1,1,1,0,0,0,1,1,1],
    [1,1,1,1,1,1,0,0,0,1,1,1],
    [1,1,1,1,1,0,1,1,1,0,0,0],
    [1,1,1,1,1,0,1,1,1,0,0,0],
    [1,1,1,1,1,0,1,1,1,0,0,0]], dtype=np.float32)


def host_consts():
    bo = np.kron(np.eye(G_PER_TILE, dtype=np.float32),
                 np.ones((N, N), dtype=np.float32))           # [120,120]
    tid = np.tile(np.eye(N, dtype=np.float32), (G_PER_TILE, 1))   # [120,12]
    adjb = np.tile(ADJ_BASE, (G_PER_TILE, ST_TILES))              # [120,96]
    idm = np.tile(np.eye(N, dtype=np.float32), (G_PER_TILE, ST_TILES))  # [120,96]
    i120 = np.eye(TILE_R, dtype=np.float32)                       # [120,120]
    return {
        "bo": bo.astype(NPBF16),
        "tid": tid.astype(NPBF16),
        "adjb": adjb.astype(np.float32),
        "idm": idm.astype(np.float32),
        "i120": i120.astype(NPBF16),
    }


def build_nc(n_tiles: int):
    """Build the per-core Bass program for n_tiles tiles of TILE_R rows."""
    rows = n_tiles * TILE_R
    rows_x = rows + 8        # transpose loads read [row0, row0+128)
    nc = bacc.Bacc(get_trn_type() or "TRN2", target_bir_lowering=False)
    nc.detect_race_conditions = False

    x_d = nc.declare_dram_parameter("x_bf", [rows_x, C], BF16, False)
    am_d = nc.declare_dram_parameter("adj", [rows, N], F32, False)
    w_d = nc.declare_dram_parameter("w_bf", [C, C], BF16, False)
    wlr_d = nc.declare_dram_parameter("wlr_bf", [C, 2], BF16, False)
    bo_d = nc.declare_dram_parameter("bo", [TILE_R, TILE_R], BF16, False)
    tid_d = nc.declare_dram_parameter("tid", [TILE_R, N], BF16, False)
    adjb_d = nc.declare_dram_parameter("adjb", [TILE_R, N * ST_TILES], F32, False)
    idm_d = nc.declare_dram_parameter("idm", [TILE_R, N * ST_TILES], F32, False)
    i120_d = nc.declare_dram_parameter("i120", [TILE_R, TILE_R], BF16, False)
    out_d = nc.declare_dram_parameter("out", [rows, C], F32, True)

    with ExitStack() as ctx:
        tc = ctx.enter_context(tile.TileContext(nc))
        _body(ctx, tc, n_tiles, x_d, am_d, w_d, wlr_d,
              bo_d, tid_d, adjb_d, idm_d, i120_d, out_d)
    nc.compile()
    return nc


def _body(ctx, tc, n_tiles, x_d, am_d, w_d, wlr_d,
          bo_d, tid_d, adjb_d, idm_d, i120_d, out_d):
    nc = tc.nc
    JW = N * ST_TILES   # 96

    cpool = ctx.enter_context(tc.tile_pool(name="consts", bufs=1))
    # resident weights / constants
    w_sb = cpool.tile([128, KC * C], BF16, name="w_sb")
    wlr_sb = cpool.tile([128, KC * 2], BF16, name="wlr_sb")
    for k in range(KC):
        nc.sync.dma_start(w_sb[:, k * C:(k + 1) * C], w_d[128 * k:128 * (k + 1), :])
        nc.sync.dma_start(wlr_sb[:, 2 * k:2 * k + 2], wlr_d[128 * k:128 * (k + 1), :])
    bo_sb = cpool.tile([TILE_R, TILE_R], BF16, name="bo_sb")
    nc.sync.dma_start(bo_sb[:], bo_d[:])
    tid_sb = cpool.tile([TILE_R, N], BF16, name="tid_sb")
    nc.sync.dma_start(tid_sb[:], tid_d[:])
    adjb_sb = cpool.tile([TILE_R, JW], F32, name="adjb_sb")
    nc.sync.dma_start(adjb_sb[:], adjb_d[:])
    idm_sb = cpool.tile([TILE_R, JW], F32, name="idm_sb")
    nc.sync.dma_start(idm_sb[:], idm_d[:])
    i120_sb = cpool.tile([TILE_R, TILE_R], BF16, name="i120_sb")
    nc.sync.dma_start(i120_sb[:], i120_d[:])

    # persistent block-diagonal alpha tiles (off-diagonal zeros written once)
    NBD = 3
    bd_tiles = []
    for bi in range(NBD):
        bdt = cpool.tile([TILE_R, TILE_R], BF16, name=f"bd{bi}_sb")
        nc.vector.memset(bdt[:], 0.0)
        bd_tiles.append(bdt)

    xn_pool = ctx.enter_context(tc.tile_pool(name="xn", bufs=12))
    xt_pool = ctx.enter_context(tc.tile_pool(name="xt", bufs=4))
    h_pool = ctx.enter_context(tc.tile_pool(name="h", bufs=12))
    o_pool = ctx.enter_context(tc.tile_pool(name="o", bufs=4))
    at_pool = ctx.enter_context(tc.tile_pool(name="attn", bufs=2))
    ph_pool = ctx.enter_context(tc.tile_pool(name="ph", bufs=2, space="PSUM"))
    pg_pool = ctx.enter_context(tc.tile_pool(name="pg", bufs=2, space="PSUM"))
    pe_pool = ctx.enter_context(tc.tile_pool(name="pe", bufs=2, space="PSUM"))
    pb_pool = ctx.enter_context(tc.tile_pool(name="pb", bufs=1, space="PSUM"))
    pt_pool = ctx.enter_context(tc.tile_pool(name="pt", bufs=1, space="PSUM"))

    n_st = (n_tiles + ST_TILES - 1) // ST_TILES
    bd_i = 0
    for st in range(n_st):
        t0 = st * ST_TILES
        nt = min(ST_TILES, n_tiles - t0)
        jw = N * nt

        # adjacency rows for the whole super-tile: [120, nt, 12]
        am_sup = at_pool.tile([TILE_R, JW], F32, tag="am")
        am_src = am_d[:].rearrange("(T p) j -> T p j", p=TILE_R)[t0:t0 + nt]
        nc.sync.dma_start(
            am_sup[:].rearrange("p (T j) -> p T j", j=N)[:, 0:nt],
            am_src.transpose([1, 0, 2]))

        elr_ps = pe_pool.tile([128, 2 * ST_TILES], F32, tag="elr")
        h_tiles = []
        xn_tiles = []
        for t in range(nt):
            row0 = (t0 + t) * TILE_R
            xn = xn_pool.tile([TILE_R, C], BF16, tag="xn")
            nc.sync.dma_start(xn[:], x_d[row0:row0 + TILE_R, :])
            xn_tiles.append(xn)

            xt = xt_pool.tile([128, KC * 128], BF16, tag="xt")
            for k in range(KC):
                nc.sync.dma_start(
                    out=xt[:, 128 * k:128 * (k + 1)],
                    in_=x_d[row0:row0 + 128, 128 * k:128 * (k + 1)],
                    transpose=True)

            ph = ph_pool.tile([128, C], F32, tag="ph")
            for k in range(KC):
                lhsT = xt[:, 128 * k:128 * (k + 1)]
                nc.tensor.matmul(ph[:], lhsT, w_sb[:, k * C:(k + 1) * C],
                                 start=(k == 0), stop=(k == KC - 1))
                nc.tensor.matmul(elr_ps[:, 2 * t:2 * t + 2], lhsT,
                                 wlr_sb[:, 2 * k:2 * k + 2],
                                 start=(k == 0), stop=(k == KC - 1))
            h_sb = h_pool.tile([TILE_R, C], BF16, tag="h")
            nc.scalar.copy(h_sb[:], ph[0:TILE_R, :])
            h_tiles.append(h_sb)

        # --- attention chain on [120, nt*12] ---
        # rhs_tid[p=(g,j'), (t,j)] = er_t[(g,j')] * (j'==j)
        rhs_tid = at_pool.tile([TILE_R, JW], BF16, tag="rhs_tid")
        tid3 = tid_sb[:].unsqueeze(1).broadcast_to([TILE_R, nt, N])
        er3 = elr_ps[0:TILE_R, 1:2 * nt:2].unsqueeze(2).broadcast_to([TILE_R, nt, N])
        nc.vector.tensor_tensor(
            rhs_tid[:].rearrange("p (T j) -> p T j", j=N)[:, 0:nt],
            tid3, er3, mybir.AluOpType.mult)

        # er_bcast[p=(g,i), (t,j)] = er_t[(g,j)]  via block-ones matmul
        eb_ps = pb_pool.tile([TILE_R, JW], F32, tag="eb")
        nc.tensor.matmul(eb_ps[:, 0:jw], bo_sb[:], rhs_tid[:, 0:jw],
                         start=True, stop=True)

        el8 = at_pool.tile([TILE_R, ST_TILES], F32, tag="el8")
        nc.vector.tensor_copy(el8[:, 0:nt], elr_ps[0:TILE_R, 0:2 * nt:2])

        # e = el + er_bcast ; e2 = lrelu(e)
        e_sb = at_pool.tile([TILE_R, JW], F32, tag="e_sb")
        el3 = el8[:, 0:nt].unsqueeze(2).broadcast_to([TILE_R, nt, N])
        nc.vector.tensor_tensor(
            e_sb[:].rearrange("p (T j) -> p T j", j=N)[:, 0:nt],
            eb_ps[:, 0:jw].rearrange("p (T j) -> p T j", j=N),
            el3, mybir.AluOpType.add)
        e2 = at_pool.tile([TILE_R, JW], F32, tag="e2")
        nc.vector.scalar_tensor_tensor(
            e2[:, 0:jw], e_sb[:, 0:jw], NEG_SLOPE, e_sb[:, 0:jw],
            mybir.AluOpType.mult, mybir.AluOpType.max)

        # pass = (adj_mask > 0.1)*ADJ_BASE + I
        q = at_pool.tile([TILE_R, JW], F32, tag="q")
        nc.vector.scalar_tensor_tensor(
            q[:, 0:jw], am_sup[:, 0:jw], THRED, adjb_sb[:, 0:jw],
            mybir.AluOpType.is_gt, mybir.AluOpType.mult)
        pass_ = at_pool.tile([TILE_R, JW], F32, tag="pass")
        nc.vector.tensor_tensor(pass_[:, 0:jw], q[:, 0:jw], idm_sb[:, 0:jw],
                                mybir.AluOpType.add)

        expv = at_pool.tile([TILE_R, JW], F32, tag="expv")
        nc.scalar.activation(expv[:, 0:jw], e2[:, 0:jw],
                             mybir.ActivationFunctionType.Exp)

        alphau = at_pool.tile([TILE_R, JW], BF16, tag="alphau")
        nc.vector.tensor_tensor(alphau[:, 0:jw], expv[:, 0:jw], pass_[:, 0:jw],
                                mybir.AluOpType.mult)

        s8 = at_pool.tile([TILE_R, ST_TILES], F32, tag="s8")
        nc.vector.tensor_reduce(
            s8[:, 0:nt],
            alphau[:].rearrange("p (T j) -> p T j", j=N)[:, 0:nt],
            mybir.AxisListType.X, mybir.AluOpType.add)
        recip8 = at_pool.tile([TILE_R, ST_TILES], F32, tag="recip8")
        nc.vector.reciprocal(recip8[:, 0:nt], s8[:, 0:nt])

        # transpose alpha: [120, nt*12] -> [nt*12, 120]
        paT = pt_pool.tile([JW, TILE_R], BF16, tag="paT")
        nc.tensor.matmul(paT[0:jw, :], alphau[:, 0:jw], i120_sb[:],
                         is_transpose=True)
        aT_sb = at_pool.tile([JW, TILE_R], BF16, tag="aT_sb")
        if nt < ST_TILES:
            nc.vector.memset(aT_sb[:], 0.0)
        nc.scalar.copy(aT_sb[0:jw, :], paT[0:jw, :])

        for t in range(nt):
            row0 = (t0 + t) * TILE_R
            # scatter alpha_t^T blocks onto the block diagonal of bd
            bd = bd_tiles[bd_i]
            bd_ap = bd[:]
            for g in range(G_PER_TILE):
                nc.gpsimd.dma_start(
                    out=bd[g * N:(g + 1) * N, g * N:(g + 1) * N],
                    in_=aT_sb[N * t:N * (t + 1), g * N:(g + 1) * N])

            pagg = pg_pool.tile([TILE_R, C], F32, tag="pagg")
            nc.tensor.matmul(pagg[:], bd_ap, h_tiles[t][:], start=True, stop=True)

            out_sb = o_pool.tile([TILE_R, C], F32, tag="out_sb")
            nc.vector.scalar_tensor_tensor(
                out_sb[:], pagg[:], recip8[:, t:t + 1], xn_tiles[t][:],
                mybir.AluOpType.mult, mybir.AluOpType.add)
            nc.sync.dma_start(out_d[row0:row0 + TILE_R, :], out_sb[:])
            bd_i = (bd_i + 1) % NBD


_NC_CACHE = {}


def _get_nc(n_tiles):
    if n_tiles not in _NC_CACHE:
        _NC_CACHE[n_tiles] = build_nc(n_tiles)
    return _NC_CACHE[n_tiles]


def prep_core_inputs(x, adj_mask, W, a_l, a_r):
    """Host-side prep: cast, pad, shard. Returns (in_maps, rows_real)."""
    B = x.shape[0]
    assert B % N_CORES == 0
    bpc = B // N_CORES
    rows_real = bpc * N
    n_tiles = (rows_real + TILE_R - 1) // TILE_R
    rows = n_tiles * TILE_R
    rows_x = rows + 8

    Wf = np.asarray(W, dtype=np.float32)
    wl = Wf @ np.asarray(a_l, dtype=np.float32)
    wr = Wf @ np.asarray(a_r, dtype=np.float32)
    w_bf = Wf.astype(NPBF16)
    wlr_bf = np.stack([wl, wr], axis=1).astype(NPBF16)
    consts = host_consts()

    x_bf_full = np.asarray(x, dtype=np.float32).astype(NPBF16)
    adj_full = np.asarray(adj_mask, dtype=np.float32)

    in_maps = []
    for c in range(N_CORES):
        xs = x_bf_full[c * bpc:(c + 1) * bpc].reshape(rows_real, C)
        xp = np.zeros((rows_x, C), dtype=NPBF16)
        xp[:rows_real] = xs
        ams = adj_full[c * bpc:(c + 1) * bpc].reshape(rows_real, N)
        amp = np.zeros((rows, N), dtype=np.float32)
        amp[:rows_real] = ams
        in_maps.append({
            "x_bf": xp, "adj": amp, "w_bf": w_bf, "wlr_bf": wlr_bf,
            "bo": consts["bo"], "tid": consts["tid"], "adjb": consts["adjb"],
            "idm": consts["idm"], "i120": consts["i120"],
        })
    return in_maps, rows_real, n_tiles


def kernel(x, adj_mask, W, a_l, a_r):
    x = np.asarray(x)
    in_dtype = x.dtype
    B = x.shape[0]
    in_maps, rows_real, n_tiles = prep_core_inputs(x, adj_mask, W, a_l, a_r)
    nc = _get_nc(n_tiles)
    res = run_bass_kernel_spmd(nc, in_maps, list(range(N_CORES)))
    bpc = B // N_CORES
    outs = [np.asarray(res.results[c]["out"][:rows_real]).reshape(bpc, N, C)
            for c in range(N_CORES)]
    return np.concatenate(outs, axis=0).astype(in_dtype, copy=False)



# revision 6
# speedup vs baseline: 5665.0917x; 5665.0917x over previous
"""GAT message-passing kernel for Trainium2 (8 NeuronCores, batch data-parallel).

out[b,i,:] = sum_j softmax_j(mask(leaky_relu(el_i + er_j))) * h[b,j,:] + x[b,i,:]
  h = x @ W, el = x @ (W a_l), er = x @ (W a_r)
  mask: ADJ_BASE*adj_mask + I > 0.1

Layout: rows (b,n) flattened; tiles of 120 rows = 10 graphs; 8 tiles form one
"super-tile" for the attention elementwise chain ([120, 96] ops).

v2 changes vs baseline:
- output stored bf16 (halves store traffic)
- per-supertile batched DMA: one natural x load, 4 batched transpose loads,
  one adj load (host pre-laid-out), one out store
- block-diag alpha built via replicate-matmul (tidT @ aT) + block-ones mask
  instead of 10 tiny SBUF->SBUF DMAs per tile
"""

import numpy as np
import ml_dtypes
from contextlib import ExitStack

import concourse.bass as bass
import concourse.bacc as bacc
import concourse.tile as tile
from concourse import mybir
from concourse.ap import AP
from concourse.bass_utils import run_bass_kernel_spmd
from concourse.bass_test_utils import get_trn_type

N = 12
C = 512
KC = C // 128            # 4 contraction chunks
NEG_SLOPE = 0.2
THRED = 0.1
N_CORES = 8
TILE_R = 120             # rows per matmul tile (10 graphs)
G_PER_TILE = TILE_R // N
ST_TILES = 8             # tiles per super-tile
JW = N * ST_TILES        # 96
XT_COLS = 1024           # per-k-chunk column block in the transposed x tile
BF16 = mybir.dt.bfloat16
F32 = mybir.dt.float32
NPBF16 = ml_dtypes.bfloat16

ADJ_BASE = np.array([
    [0,0,0,1,0,1,1,1,1,1,1,1],
    [0,0,0,1,0,1,1,1,1,1,1,1],
    [0,0,0,1,0,1,1,1,1,1,1,1],
    [1,1,1,0,1,1,1,1,1,1,1,1],
    [0,0,0,1,0,1,1,1,1,1,1,1],
    [1,1,1,1,1,0,1,1,1,0,0,0],
    [1,1,1,1,1,1,0,0,0,1,1,1],
    [1,1,1,1,1,1,0,0,0,1,1,1],
    [1,1,1,1,1,1,0,0,0,1,1,1],
    [1,1,1,1,1,0,1,1,1,0,0,0],
    [1,1,1,1,1,0,1,1,1,0,0,0],
    [1,1,1,1,1,0,1,1,1,0,0,0]], dtype=np.float32)


def host_consts():
    bo = np.kron(np.eye(G_PER_TILE, dtype=np.float32),
                 np.ones((N, N), dtype=np.float32))           # [120,120]
    tidT = np.tile(np.eye(N, dtype=np.float32), (1, G_PER_TILE))  # [12,120]
    adjb = np.tile(ADJ_BASE, (G_PER_TILE, ST_TILES))              # [120,96]
    idm = np.tile(np.eye(N, dtype=np.float32), (G_PER_TILE, ST_TILES))  # [120,96]
    i120 = np.eye(TILE_R, dtype=np.float32)                       # [120,120]
    return {
        "bo": bo.astype(NPBF16),
        "tidT": tidT.astype(NPBF16),
        "adjb": adjb.astype(np.float32),
        "idm": idm.astype(np.float32),
        "i120": i120.astype(NPBF16),
    }


def build_nc(n_tiles: int):
    """Build the per-core Bass program for n_tiles tiles of TILE_R rows."""
    rows = n_tiles * TILE_R
    rows_x = rows + 64       # transposed loads read up to ceil(nt*120/128)*128
    n_st = (n_tiles + ST_TILES - 1) // ST_TILES
    nc = bacc.Bacc(get_trn_type() or "TRN2", target_bir_lowering=False)
    nc.detect_race_conditions = False

    x_d = nc.declare_dram_parameter("x_bf", [rows_x, C], BF16, False)
    am_d = nc.declare_dram_parameter("adj", [n_st * TILE_R, JW], F32, False)
    w_d = nc.declare_dram_parameter("w_bf", [C, C], BF16, False)
    wlr_d = nc.declare_dram_parameter("wlr_bf", [C, 2], BF16, False)
    bo_d = nc.declare_dram_parameter("bo", [TILE_R, TILE_R], BF16, False)
    tidT_d = nc.declare_dram_parameter("tidT", [N, TILE_R], BF16, False)
    adjb_d = nc.declare_dram_parameter("adjb", [TILE_R, JW], F32, False)
    idm_d = nc.declare_dram_parameter("idm", [TILE_R, JW], F32, False)
    i120_d = nc.declare_dram_parameter("i120", [TILE_R, TILE_R], BF16, False)
    out_d = nc.declare_dram_parameter("out", [rows, C], BF16, True)

    with ExitStack() as ctx:
        tc = ctx.enter_context(tile.TileContext(nc))
        _body(ctx, tc, n_tiles, x_d, am_d, w_d, wlr_d,
              bo_d, tidT_d, adjb_d, idm_d, i120_d, out_d)
    nc.compile()
    return nc


def _body(ctx, tc, n_tiles, x_d, am_d, w_d, wlr_d,
          bo_d, tidT_d, adjb_d, idm_d, i120_d, out_d):
    nc = tc.nc

    cpool = ctx.enter_context(tc.tile_pool(name="consts", bufs=1))
    # resident weights / constants
    w_sb = cpool.tile([128, KC * C], BF16, name="w_sb")
    wlr_sb = cpool.tile([128, KC * 2], BF16, name="wlr_sb")
    for k in range(KC):
        nc.sync.dma_start(w_sb[:, k * C:(k + 1) * C], w_d[128 * k:128 * (k + 1), :])
        nc.sync.dma_start(wlr_sb[:, 2 * k:2 * k + 2], wlr_d[128 * k:128 * (k + 1), :])
    bo_sb = cpool.tile([TILE_R, TILE_R], BF16, name="bo_sb")
    nc.sync.dma_start(bo_sb[:], bo_d[:])
    tidT_sb = cpool.tile([N, TILE_R], BF16, name="tidT_sb")
    nc.sync.dma_start(tidT_sb[:], tidT_d[:])
    adjb_sb = cpool.tile([TILE_R, JW], F32, name="adjb_sb")
    nc.sync.dma_start(adjb_sb[:], adjb_d[:])
    idm_sb = cpool.tile([TILE_R, JW], F32, name="idm_sb")
    nc.sync.dma_start(idm_sb[:], idm_d[:])
    i120_sb = cpool.tile([TILE_R, TILE_R], BF16, name="i120_sb")
    nc.sync.dma_start(i120_sb[:], i120_d[:])

    xn_pool = ctx.enter_context(tc.tile_pool(name="xn", bufs=2))
    xt_pool = ctx.enter_context(tc.tile_pool(name="xt", bufs=2))
    h_pool = ctx.enter_context(tc.tile_pool(name="h", bufs=12))
    o_pool = ctx.enter_context(tc.tile_pool(name="o", bufs=2))
    at_pool = ctx.enter_context(tc.tile_pool(name="attn", bufs=2))
    bd_pool = ctx.enter_context(tc.tile_pool(name="bd", bufs=3))
    ph_pool = ctx.enter_context(tc.tile_pool(name="ph", bufs=2, space="PSUM"))
    pe_pool = ctx.enter_context(tc.tile_pool(name="pe", bufs=2, space="PSUM"))
    ps_pool = ctx.enter_context(tc.tile_pool(name="ps", bufs=2, space="PSUM"))
    pg_pool = ctx.enter_context(tc.tile_pool(name="pg", bufs=2, space="PSUM"))

    n_st = (n_tiles + ST_TILES - 1) // ST_TILES
    for st in range(n_st):
        t0 = st * ST_TILES
        nt = min(ST_TILES, n_tiles - t0)
        jw = N * nt
        r0 = t0 * TILE_R
        xt_free = ((nt * TILE_R + 127) // 128) * 128   # 1024 (nt=8) / 640 (nt=5)

        # ---- batched DMAs for the super-tile ----
        am_sup = at_pool.tile([TILE_R, JW], F32, tag="am")
        nc.sync.dma_start(am_sup[:, 0:jw],
                          am_d[st * TILE_R:(st + 1) * TILE_R, 0:jw])

        xn_sup = xn_pool.tile([TILE_R, ST_TILES * C], BF16, tag="xn")
        xn_src = x_d[:].rearrange("(T p) c -> T p c", p=TILE_R)[t0:t0 + nt]
        nc.sync.dma_start(
            xn_sup[:].rearrange("p (T c) -> p T c", c=C)[:, 0:nt],
            xn_src.transpose([1, 0, 2]))

        xt_sup = xt_pool.tile([128, KC * XT_COLS], BF16, tag="xt")
        for k in range(KC):
            nc.sync.dma_start(
                out=xt_sup[:, k * XT_COLS:k * XT_COLS + xt_free],
                in_=x_d[r0:r0 + xt_free, 128 * k:128 * (k + 1)],
                transpose=True)

        # ---- projection matmuls: h = x@W, elr = x@[wl wr] ----
        elr_ps = pe_pool.tile([TILE_R, 2 * ST_TILES], F32, tag="elr")
        h_tiles = []
        for t in range(nt):
            ph = ph_pool.tile([TILE_R, C], F32, tag="ph")
            for k in range(KC):
                lhsT = xt_sup[:, k * XT_COLS + t * TILE_R:
                              k * XT_COLS + (t + 1) * TILE_R]
                nc.tensor.matmul(ph[:], lhsT, w_sb[:, k * C:(k + 1) * C],
                                 start=(k == 0), stop=(k == KC - 1))
                nc.tensor.matmul(elr_ps[:, 2 * t:2 * t + 2], lhsT,
                                 wlr_sb[:, 2 * k:2 * k + 2],
                                 start=(k == 0), stop=(k == KC - 1))
            h_sb = h_pool.tile([TILE_R, C], BF16, tag="h")
            nc.scalar.copy(h_sb[:], ph[:])
            h_tiles.append(h_sb)

        # free the elr PSUM bank early: copy to SBUF
        elr_sb = at_pool.tile([TILE_R, 2 * ST_TILES], F32, tag="elr_sb")
        nc.vector.tensor_copy(elr_sb[:, 0:2 * nt], elr_ps[:, 0:2 * nt])

        # ---- attention chain on [120, nt*12] ----
        # rhs_tid[p=(g,j'), (t,j)] = er_t[(g,j')] * (j'==j)
        rhs_tid = at_pool.tile([TILE_R, JW], BF16, tag="rhs_tid")
        idm3 = idm_sb[:].rearrange("p (T j) -> p T j", j=N)[:, 0:nt]
        er3 = elr_sb[:, 1:2 * nt:2].unsqueeze(2).broadcast_to([TILE_R, nt, N])
        nc.vector.tensor_tensor(
            rhs_tid[:].rearrange("p (T j) -> p T j", j=N)[:, 0:nt],
            idm3, er3, mybir.AluOpType.mult)

        # er_bcast[p=(g,i), (t,j)] = er_t[(g,j)]  via block-ones matmul
        eb_ps = ps_pool.tile([TILE_R, JW], F32, tag="small")
        nc.tensor.matmul(eb_ps[:, 0:jw], bo_sb[:], rhs_tid[:, 0:jw],
                         start=True, stop=True)

        el8 = at_pool.tile([TILE_R, ST_TILES], F32, tag="el8")
        nc.vector.tensor_copy(el8[:, 0:nt], elr_sb[:, 0:2 * nt:2])

        # e = el + er_bcast ; e2 = lrelu(e)
        e_sb = at_pool.tile([TILE_R, JW], F32, tag="e_sb")
        el3 = el8[:, 0:nt].unsqueeze(2).broadcast_to([TILE_R, nt, N])
        nc.vector.tensor_tensor(
            e_sb[:].rearrange("p (T j) -> p T j", j=N)[:, 0:nt],
            eb_ps[:, 0:jw].rearrange("p (T j) -> p T j", j=N),
            el3, mybir.AluOpType.add)
        e2 = at_pool.tile([TILE_R, JW], F32, tag="e2")
        nc.vector.scalar_tensor_tensor(
            e2[:, 0:jw], e_sb[:, 0:jw], NEG_SLOPE, e_sb[:, 0:jw],
            mybir.AluOpType.mult, mybir.AluOpType.max)

        # pass = (adj_mask > 0.1)*ADJ_BASE + I
        q = at_pool.tile([TILE_R, JW], F32, tag="q")
        nc.vector.scalar_tensor_tensor(
            q[:, 0:jw], am_sup[:, 0:jw], THRED, adjb_sb[:, 0:jw],
            mybir.AluOpType.is_gt, mybir.AluOpType.mult)
        pass_ = at_pool.tile([TILE_R, JW], F32, tag="pass")
        nc.vector.tensor_tensor(pass_[:, 0:jw], q[:, 0:jw], idm_sb[:, 0:jw],
                                mybir.AluOpType.add)

        expv = at_pool.tile([TILE_R, JW], F32, tag="expv")
        nc.scalar.activation(expv[:, 0:jw], e2[:, 0:jw],
                             mybir.ActivationFunctionType.Exp)

        alphau = at_pool.tile([TILE_R, JW], BF16, tag="alphau")
        nc.vector.tensor_tensor(alphau[:, 0:jw], expv[:, 0:jw], pass_[:, 0:jw],
                                mybir.AluOpType.mult)

        s8 = at_pool.tile([TILE_R, ST_TILES], F32, tag="s8")
        nc.vector.tensor_reduce(
            s8[:, 0:nt],
            alphau[:].rearrange("p (T j) -> p T j", j=N)[:, 0:nt],
            mybir.AxisListType.X, mybir.AluOpType.add)
        recip8 = at_pool.tile([TILE_R, ST_TILES], F32, tag="recip8")
        nc.vector.reciprocal(recip8[:, 0:nt], s8[:, 0:nt])

        # transpose alpha: [120, nt*12] -> [nt*12, 120]
        paT = ps_pool.tile([JW, TILE_R], BF16, tag="small")
        nc.tensor.matmul(paT[0:jw, :], alphau[:, 0:jw], i120_sb[:],
                         is_transpose=True)
        aT_sb = at_pool.tile([JW, TILE_R], BF16, tag="aT_sb")
        nc.scalar.copy(aT_sb[0:jw, :], paT[0:jw, :])

        # ---- per-tile: build block-diag alpha^T, aggregate, combine ----
        out_sup = o_pool.tile([TILE_R, ST_TILES * C], BF16, tag="out_sup")
        for t in range(nt):
            # bdT replicate: bdrep[(g,i),(g',j)] = alpha[(g',j),(t,i)] then
            # mask to the block diagonal -> bdT[(g,j),(g,i)] layout for agg
            bdrep = ps_pool.tile([TILE_R, TILE_R], F32, tag="small")
            nc.tensor.matmul(bdrep[:], tidT_sb[:], aT_sb[N * t:N * (t + 1), :],
                             start=True, stop=True)
            bd_sb = bd_pool.tile([TILE_R, TILE_R], BF16, tag="bd")
            nc.vector.tensor_tensor(bd_sb[:], bdrep[:], bo_sb[:],
                                    mybir.AluOpType.mult)

            pagg = pg_pool.tile([TILE_R, C], F32, tag="pagg")
            nc.tensor.matmul(pagg[:], bd_sb[:], h_tiles[t][:],
                             start=True, stop=True)

            nc.vector.scalar_tensor_tensor(
                out_sup[:, t * C:(t + 1) * C], pagg[:], recip8[:, t:t + 1],
                xn_sup[:, t * C:(t + 1) * C],
                mybir.AluOpType.mult, mybir.AluOpType.add)

        out_dst = out_d[:].rearrange("(T p) c -> T p c", p=TILE_R)[t0:t0 + nt]
        nc.sync.dma_start(
            out_dst.transpose([1, 0, 2]),
            out_sup[:].rearrange("p (T c) -> p T c", c=C)[:, 0:nt])


_NC_CACHE = {}


def _get_nc(n_tiles):
    if n_tiles not in _NC_CACHE:
        _NC_CACHE[n_tiles] = build_nc(n_tiles)
    return _NC_CACHE[n_tiles]


def prep_core_inputs(x, adj_mask, W, a_l, a_r):
    """Host-side prep: cast, pad, shard. Returns (in_maps, rows_real, n_tiles)."""
    B = x.shape[0]
    assert B % N_CORES == 0
    bpc = B // N_CORES
    rows_real = bpc * N
    n_tiles = (rows_real + TILE_R - 1) // TILE_R
    rows = n_tiles * TILE_R
    rows_x = rows + 64
    n_st = (n_tiles + ST_TILES - 1) // ST_TILES

    Wf = np.asarray(W, dtype=np.float32)
    wl = Wf @ np.asarray(a_l, dtype=np.float32)
    wr = Wf @ np.asarray(a_r, dtype=np.float32)
    w_bf = Wf.astype(NPBF16)
    wlr_bf = np.stack([wl, wr], axis=1).astype(NPBF16)
    consts = host_consts()

    x_bf_full = np.asarray(x, dtype=np.float32).astype(NPBF16)
    adj_full = np.asarray(adj_mask, dtype=np.float32)

    in_maps = []
    for c in range(N_CORES):
        xs = x_bf_full[c * bpc:(c + 1) * bpc].reshape(rows_real, C)
        xp = np.zeros((rows_x, C), dtype=NPBF16)
        xp[:rows_real] = xs
        ams = adj_full[c * bpc:(c + 1) * bpc].reshape(rows_real, N)
        # super-tile layout: amp[st*120 + p, t*12 + j] = adj[(st*8+t)*120 + p, j]
        amp = np.zeros((n_st * ST_TILES * TILE_R, N), dtype=np.float32)
        amp[:rows_real] = ams
        amp = amp.reshape(n_st, ST_TILES, TILE_R, N).transpose(0, 2, 1, 3)
        amp = np.ascontiguousarray(amp).reshape(n_st * TILE_R, ST_TILES * N)
        in_maps.append({
            "x_bf": xp, "adj": amp, "w_bf": w_bf, "wlr_bf": wlr_bf,
            "bo": consts["bo"], "tidT": consts["tidT"], "adjb": consts["adjb"],
            "idm": consts["idm"], "i120": consts["i120"],
        })
    return in_maps, rows_real, n_tiles


def kernel(x, adj_mask, W, a_l, a_r):
    x = np.asarray(x)
    B = x.shape[0]
    in_maps, rows_real, n_tiles = prep_core_inputs(x, adj_mask, W, a_l, a_r)
    nc = _get_nc(n_tiles)
    res = run_bass_kernel_spmd(nc, in_maps, list(range(N_CORES)))
    bpc = B // N_CORES
    outs = [np.asarray(res.results[c]["out"][:rows_real]).reshape(bpc, N, C)
            for c in range(N_CORES)]
    return np.concatenate(outs, axis=0).astype(np.float32, copy=False)


# revision 9
# speedup vs baseline: 34106.8386x; 6.0205x over previous
"""GAT message-passing kernel for Trainium2 (8 NeuronCores, batch data-parallel).

out[b,i,:] = sum_j softmax_j(mask(leaky_relu(el_i + er_j))) * h[b,j,:] + x[b,i,:]
  h = x @ W, el = x @ (W a_l), er = x @ (W a_r)
  mask: ADJ_BASE*adj_mask + I > 0.1

Layout: rows (b,n) flattened; tiles of 120 rows = 10 graphs; 8 tiles form one
"super-tile" for the attention elementwise chain ([120, 96] ops).

v2 changes vs baseline:
- output stored bf16 (halves store traffic)
- per-supertile batched DMA: one natural x load, 4 batched transpose loads,
  one adj load (host pre-laid-out), one out store
- block-diag alpha built via replicate-matmul (tidT @ aT) + block-ones mask
  instead of 10 tiny SBUF->SBUF DMAs per tile
"""

import numpy as np
import ml_dtypes
from contextlib import ExitStack

import concourse.bass as bass
import concourse.bacc as bacc
import concourse.tile as tile
from concourse import mybir
from concourse.ap import AP
from concourse.bass_utils import run_bass_kernel_spmd
from concourse.bass_test_utils import get_trn_type

N = 12
C = 512
KC = C // 128            # 4 contraction chunks
NEG_SLOPE = 0.2
THRED = 0.1
N_CORES = 8
TILE_R = 120             # rows per matmul tile (10 graphs)
G_PER_TILE = TILE_R // N
ST_TILES = 8             # tiles per super-tile
JW = N * ST_TILES        # 96
XT_COLS = 1024           # per-k-chunk column block in the transposed x tile
BF16 = mybir.dt.bfloat16
F32 = mybir.dt.float32
NPBF16 = ml_dtypes.bfloat16

ADJ_BASE = np.array([
    [0,0,0,1,0,1,1,1,1,1,1,1],
    [0,0,0,1,0,1,1,1,1,1,1,1],
    [0,0,0,1,0,1,1,1,1,1,1,1],
    [1,1,1,0,1,1,1,1,1,1,1,1],
    [0,0,0,1,0,1,1,1,1,1,1,1],
    [1,1,1,1,1,0,1,1,1,0,0,0],
    [1,1,1,1,1,1,0,0,0,1,1,1],
    [1,1,1,1,1,1,0,0,0,1,1,1],
    [1,1,1,1,1,1,0,0,0,1,1,1],
    [1,1,1,1,1,0,1,1,1,0,0,0],
    [1,1,1,1,1,0,1,1,1,0,0,0],
    [1,1,1,1,1,0,1,1,1,0,0,0]], dtype=np.float32)


def host_consts():
    bo = np.kron(np.eye(G_PER_TILE, dtype=np.float32),
                 np.ones((N, N), dtype=np.float32))           # [120,120]
    tidT = np.tile(np.eye(N, dtype=np.float32), (1, G_PER_TILE))  # [12,120]
    adjb = np.tile(ADJ_BASE, (G_PER_TILE, ST_TILES))              # [120,96]
    idm = np.tile(np.eye(N, dtype=np.float32), (G_PER_TILE, ST_TILES))  # [120,96]
    i120 = np.eye(TILE_R, dtype=np.float32)                       # [120,120]
    return {
        "bo": bo.astype(NPBF16),
        "tidT": tidT.astype(NPBF16),
        "adjb": adjb.astype(np.float32),
        "idm": idm.astype(np.float32),
        "i120": i120.astype(NPBF16),
    }


def build_nc(n_tiles: int):
    """Build the per-core Bass program for n_tiles tiles of TILE_R rows."""
    rows = n_tiles * TILE_R
    rows_x = rows + 64       # transposed loads read up to ceil(nt*120/128)*128
    n_st = (n_tiles + ST_TILES - 1) // ST_TILES
    nc = bacc.Bacc(get_trn_type() or "TRN2", target_bir_lowering=False)
    nc.detect_race_conditions = False

    x_d = nc.declare_dram_parameter("x_bf", [rows_x, C], BF16, False)
    am_d = nc.declare_dram_parameter("adj", [n_st * TILE_R, JW], F32, False)
    w_d = nc.declare_dram_parameter("w_bf", [C, C], BF16, False)
    wlr_d = nc.declare_dram_parameter("wlr_bf", [C, 2], BF16, False)
    bo_d = nc.declare_dram_parameter("bo", [TILE_R, TILE_R], BF16, False)
    tidT_d = nc.declare_dram_parameter("tidT", [N, TILE_R], BF16, False)
    adjb_d = nc.declare_dram_parameter("adjb", [TILE_R, JW], F32, False)
    idm_d = nc.declare_dram_parameter("idm", [TILE_R, JW], F32, False)
    i120_d = nc.declare_dram_parameter("i120", [TILE_R, TILE_R], BF16, False)
    out_d = nc.declare_dram_parameter("out", [rows, C], BF16, True)

    with ExitStack() as ctx:
        tc = ctx.enter_context(tile.TileContext(nc))
        _body(ctx, tc, n_tiles, x_d, am_d, w_d, wlr_d,
              bo_d, tidT_d, adjb_d, idm_d, i120_d, out_d)
    nc.compile()
    return nc


def _body(ctx, tc, n_tiles, x_d, am_d, w_d, wlr_d,
          bo_d, tidT_d, adjb_d, idm_d, i120_d, out_d):
    nc = tc.nc

    cpool = ctx.enter_context(tc.tile_pool(name="consts", bufs=1))
    # resident weights / constants
    w_sb = cpool.tile([128, KC * C], BF16, name="w_sb")
    wlr_sb = cpool.tile([128, KC * 2], BF16, name="wlr_sb")
    for k in range(KC):
        nc.sync.dma_start(w_sb[:, k * C:(k + 1) * C], w_d[128 * k:128 * (k + 1), :])
        nc.sync.dma_start(wlr_sb[:, 2 * k:2 * k + 2], wlr_d[128 * k:128 * (k + 1), :])
    bo_sb = cpool.tile([TILE_R, TILE_R], BF16, name="bo_sb")
    nc.sync.dma_start(bo_sb[:], bo_d[:])
    tidT_sb = cpool.tile([N, TILE_R], BF16, name="tidT_sb")
    nc.sync.dma_start(tidT_sb[:], tidT_d[:])
    adjb_sb = cpool.tile([TILE_R, JW], F32, name="adjb_sb")
    nc.sync.dma_start(adjb_sb[:], adjb_d[:])
    idm_sb = cpool.tile([TILE_R, JW], F32, name="idm_sb")
    nc.sync.dma_start(idm_sb[:], idm_d[:])
    i120_sb = cpool.tile([TILE_R, TILE_R], BF16, name="i120_sb")
    nc.sync.dma_start(i120_sb[:], i120_d[:])

    xn_pool = ctx.enter_context(tc.tile_pool(name="xn", bufs=2))
    xt_pool = ctx.enter_context(tc.tile_pool(name="xt", bufs=2))
    h_pool = ctx.enter_context(tc.tile_pool(name="h", bufs=12))
    o_pool = ctx.enter_context(tc.tile_pool(name="o", bufs=2))
    at_pool = ctx.enter_context(tc.tile_pool(name="attn", bufs=2))
    bd_pool = ctx.enter_context(tc.tile_pool(name="bd", bufs=3))
    ph_pool = ctx.enter_context(tc.tile_pool(name="ph", bufs=2, space="PSUM"))
    pe_pool = ctx.enter_context(tc.tile_pool(name="pe", bufs=2, space="PSUM"))
    ps_pool = ctx.enter_context(tc.tile_pool(name="ps", bufs=2, space="PSUM"))
    pg_pool = ctx.enter_context(tc.tile_pool(name="pg", bufs=2, space="PSUM"))

    n_st = (n_tiles + ST_TILES - 1) // ST_TILES
    for st in range(n_st):
        t0 = st * ST_TILES
        nt = min(ST_TILES, n_tiles - t0)
        jw = N * nt
        r0 = t0 * TILE_R
        xt_free = ((nt * TILE_R + 127) // 128) * 128   # 1024 (nt=8) / 640 (nt=5)

        # ---- batched DMAs for the super-tile ----
        am_sup = at_pool.tile([TILE_R, JW], F32, tag="am")
        nc.sync.dma_start(am_sup[:, 0:jw],
                          am_d[st * TILE_R:(st + 1) * TILE_R, 0:jw])

        xn_sup = xn_pool.tile([TILE_R, ST_TILES * C], BF16, tag="xn")
        xn_src = (x_d[0:n_tiles * TILE_R, :]
                  .rearrange("(T p) c -> T p c", p=TILE_R)[t0:t0 + nt])
        nc.sync.dma_start(
            xn_sup[:].rearrange("p (T c) -> p T c", c=C)[:, 0:nt],
            xn_src.transpose([1, 0, 2]))

        xt_sup = xt_pool.tile([128, KC * XT_COLS], BF16, tag="xt")
        for k in range(KC):
            nc.sync.dma_start(
                out=xt_sup[:, k * XT_COLS:k * XT_COLS + xt_free],
                in_=x_d[r0:r0 + xt_free, 128 * k:128 * (k + 1)],
                transpose=True)

        # ---- projection matmuls: h = x@W, elr = x@[wl wr] ----
        elr_ps = pe_pool.tile([TILE_R, 2 * ST_TILES], F32, tag="elr")
        h_tiles = []
        for t in range(nt):
            ph = ph_pool.tile([TILE_R, C], F32, tag="ph")
            for k in range(KC):
                lhsT = xt_sup[:, k * XT_COLS + t * TILE_R:
                              k * XT_COLS + (t + 1) * TILE_R]
                nc.tensor.matmul(ph[:], lhsT, w_sb[:, k * C:(k + 1) * C],
                                 start=(k == 0), stop=(k == KC - 1))
                nc.tensor.matmul(elr_ps[:, 2 * t:2 * t + 2], lhsT,
                                 wlr_sb[:, 2 * k:2 * k + 2],
                                 start=(k == 0), stop=(k == KC - 1))
            h_sb = h_pool.tile([TILE_R, C], BF16, tag="h")
            nc.scalar.copy(h_sb[:], ph[:])
            h_tiles.append(h_sb)

        # free the elr PSUM bank early: copy to SBUF
        elr_sb = at_pool.tile([TILE_R, 2 * ST_TILES], F32, tag="elr_sb")
        nc.vector.tensor_copy(elr_sb[:, 0:2 * nt], elr_ps[:, 0:2 * nt])

        # ---- attention chain on [120, nt*12] ----
        # rhs_tid[p=(g,j'), (t,j)] = er_t[(g,j')] * (j'==j)
        rhs_tid = at_pool.tile([TILE_R, JW], BF16, tag="rhs_tid")
        idm3 = idm_sb[:].rearrange("p (T j) -> p T j", j=N)[:, 0:nt]
        er3 = elr_sb[:, 1:2 * nt:2].unsqueeze(2).broadcast_to([TILE_R, nt, N])
        nc.vector.tensor_tensor(
            rhs_tid[:].rearrange("p (T j) -> p T j", j=N)[:, 0:nt],
            idm3, er3, mybir.AluOpType.mult)

        # er_bcast[p=(g,i), (t,j)] = er_t[(g,j)]  via block-ones matmul
        eb_ps = ps_pool.tile([TILE_R, JW], F32, tag="small")
        nc.tensor.matmul(eb_ps[:, 0:jw], bo_sb[:], rhs_tid[:, 0:jw],
                         start=True, stop=True)

        el8 = at_pool.tile([TILE_R, ST_TILES], F32, tag="el8")
        nc.vector.tensor_copy(el8[:, 0:nt], elr_sb[:, 0:2 * nt:2])

        # e = el + er_bcast ; e2 = lrelu(e)
        e_sb = at_pool.tile([TILE_R, JW], F32, tag="e_sb")
        el3 = el8[:, 0:nt].unsqueeze(2).broadcast_to([TILE_R, nt, N])
        nc.vector.tensor_tensor(
            e_sb[:].rearrange("p (T j) -> p T j", j=N)[:, 0:nt],
            eb_ps[:, 0:jw].rearrange("p (T j) -> p T j", j=N),
            el3, mybir.AluOpType.add)
        e2 = at_pool.tile([TILE_R, JW], F32, tag="e2")
        nc.vector.scalar_tensor_tensor(
            e2[:, 0:jw], e_sb[:, 0:jw], NEG_SLOPE, e_sb[:, 0:jw],
            mybir.AluOpType.mult, mybir.AluOpType.max)

        # pass = (adj_mask > 0.1)*ADJ_BASE + I
        q = at_pool.tile([TILE_R, JW], F32, tag="q")
        nc.vector.scalar_tensor_tensor(
            q[:, 0:jw], am_sup[:, 0:jw], THRED, adjb_sb[:, 0:jw],
            mybir.AluOpType.is_gt, mybir.AluOpType.mult)
        pass_ = at_pool.tile([TILE_R, JW], F32, tag="pass")
        nc.vector.tensor_tensor(pass_[:, 0:jw], q[:, 0:jw], idm_sb[:, 0:jw],
                                mybir.AluOpType.add)

        expv = at_pool.tile([TILE_R, JW], F32, tag="expv")
        nc.scalar.activation(expv[:, 0:jw], e2[:, 0:jw],
                             mybir.ActivationFunctionType.Exp)

        alphau = at_pool.tile([TILE_R, JW], BF16, tag="alphau")
        nc.vector.tensor_tensor(alphau[:, 0:jw], expv[:, 0:jw], pass_[:, 0:jw],
                                mybir.AluOpType.mult)

        s8 = at_pool.tile([TILE_R, ST_TILES], F32, tag="s8")
        nc.vector.tensor_reduce(
            s8[:, 0:nt],
            alphau[:].rearrange("p (T j) -> p T j", j=N)[:, 0:nt],
            mybir.AxisListType.X, mybir.AluOpType.add)
        recip8 = at_pool.tile([TILE_R, ST_TILES], F32, tag="recip8")
        nc.vector.reciprocal(recip8[:, 0:nt], s8[:, 0:nt])

        # transpose alpha per tile: [120, 12] -> [12, 120] at free offset t*120
        # (keeps every later matmul operand at base partition 0)
        paT = ps_pool.tile([N, ST_TILES * TILE_R], BF16, tag="small")
        for t in range(nt):
            nc.tensor.matmul(paT[:, t * TILE_R:(t + 1) * TILE_R],
                             alphau[:, N * t:N * (t + 1)], i120_sb[:],
                             is_transpose=True)
        aT_sb = at_pool.tile([N, ST_TILES * TILE_R], BF16, tag="aT_sb")
        nc.scalar.copy(aT_sb[:, 0:nt * TILE_R], paT[:, 0:nt * TILE_R])

        # ---- per-tile: build block-diag alpha^T, aggregate, combine ----
        out_sup = o_pool.tile([TILE_R, ST_TILES * C], BF16, tag="out_sup")
        for t in range(nt):
            # bdT replicate: bdrep[(g,i),(g',j)] = alpha[(g',j),(t,i)] then
            # mask to the block diagonal -> bdT[(g,j),(g,i)] layout for agg
            bdrep = ps_pool.tile([TILE_R, TILE_R], F32, tag="small")
            nc.tensor.matmul(bdrep[:], tidT_sb[:],
                             aT_sb[:, t * TILE_R:(t + 1) * TILE_R],
                             start=True, stop=True)
            bd_sb = bd_pool.tile([TILE_R, TILE_R], BF16, tag="bd")
            nc.vector.tensor_tensor(bd_sb[:], bdrep[:], bo_sb[:],
                                    mybir.AluOpType.mult)

            pagg = pg_pool.tile([TILE_R, C], F32, tag="pagg")
            nc.tensor.matmul(pagg[:], bd_sb[:], h_tiles[t][:],
                             start=True, stop=True)

            nc.vector.scalar_tensor_tensor(
                out_sup[:, t * C:(t + 1) * C], pagg[:], recip8[:, t:t + 1],
                xn_sup[:, t * C:(t + 1) * C],
                mybir.AluOpType.mult, mybir.AluOpType.add)

        out_dst = out_d[:].rearrange("(T p) c -> T p c", p=TILE_R)[t0:t0 + nt]
        nc.sync.dma_start(
            out_dst.transpose([1, 0, 2]),
            out_sup[:].rearrange("p (T c) -> p T c", c=C)[:, 0:nt])


_NC_CACHE = {}


def _get_nc(n_tiles):
    if n_tiles not in _NC_CACHE:
        _NC_CACHE[n_tiles] = build_nc(n_tiles)
    return _NC_CACHE[n_tiles]


def prep_core_inputs(x, adj_mask, W, a_l, a_r):
    """Host-side prep: cast, pad, shard. Returns (in_maps, rows_real, n_tiles)."""
    B = x.shape[0]
    assert B % N_CORES == 0
    bpc = B // N_CORES
    rows_real = bpc * N
    n_tiles = (rows_real + TILE_R - 1) // TILE_R
    rows = n_tiles * TILE_R
    rows_x = rows + 64
    n_st = (n_tiles + ST_TILES - 1) // ST_TILES

    Wf = np.asarray(W, dtype=np.float32)
    wl = Wf @ np.asarray(a_l, dtype=np.float32)
    wr = Wf @ np.asarray(a_r, dtype=np.float32)
    w_bf = Wf.astype(NPBF16)
    wlr_bf = np.stack([wl, wr], axis=1).astype(NPBF16)
    consts = host_consts()

    x_bf_full = np.asarray(x, dtype=np.float32).astype(NPBF16)
    adj_full = np.asarray(adj_mask, dtype=np.float32)

    in_maps = []
    for c in range(N_CORES):
        xs = x_bf_full[c * bpc:(c + 1) * bpc].reshape(rows_real, C)
        xp = np.zeros((rows_x, C), dtype=NPBF16)
        xp[:rows_real] = xs
        ams = adj_full[c * bpc:(c + 1) * bpc].reshape(rows_real, N)
        # super-tile layout: amp[st*120 + p, t*12 + j] = adj[(st*8+t)*120 + p, j]
        amp = np.zeros((n_st * ST_TILES * TILE_R, N), dtype=np.float32)
        amp[:rows_real] = ams
        amp = amp.reshape(n_st, ST_TILES, TILE_R, N).transpose(0, 2, 1, 3)
        amp = np.ascontiguousarray(amp).reshape(n_st * TILE_R, ST_TILES * N)
        in_maps.append({
            "x_bf": xp, "adj": amp, "w_bf": w_bf, "wlr_bf": wlr_bf,
            "bo": consts["bo"], "tidT": consts["tidT"], "adjb": consts["adjb"],
            "idm": consts["idm"], "i120": consts["i120"],
        })
    return in_maps, rows_real, n_tiles


def kernel(x, adj_mask, W, a_l, a_r):
    x = np.asarray(x)
    B = x.shape[0]
    in_maps, rows_real, n_tiles = prep_core_inputs(x, adj_mask, W, a_l, a_r)
    nc = _get_nc(n_tiles)
    res = run_bass_kernel_spmd(nc, in_maps, list(range(N_CORES)))
    bpc = B // N_CORES
    outs = [np.asarray(res.results[c]["out"][:rows_real]).reshape(bpc, N, C)
            for c in range(N_CORES)]
    return np.concatenate(outs, axis=0).astype(np.float32, copy=False)
